# revision 79
# baseline (speedup 1.0000x reference)
"""Trainium2 Bass kernel for nn_Decoder (3-step LSTM decoder w/ Luong attention
+ conv1d entity heads). Data-parallel over batch: B=64 -> 8 cores x 8.

Restructured so every non-conv matmul keeps its large dims on the PE
partition/stationary side and streams only a tiny output free dim (the PE
cost is out_free_size cycles): LSTM gates / scores / mix / attends / vbias /
relation logits all produce [*, batch<=8] or [*, 3] outputs; the entity-head
reduction consumes each relu tile as the stationary operand against
Went [128, 2] (2-cycle matmuls) and the per-batch result is PE-transposed
once and written with a single DMA per batch.

Decomposition (validated vs reference to 5e-7):
  - conv1d over feat=[enc, broadcast(o)] splits into a 3-tap matmul conv over
    enc (shared by both ent_heads calls) plus a per-batch bias vec (with
    first/last-column variants for the SAME-padding edges).
  - attend(q) = tanh(mix @ Wa[:, :E].T + q @ Wa[:, E:].T + b) with
    mix = softmax(q.enc) @ enc.
All heavy matmuls run in bf16 (fp32 PSUM accumulation).
"""
import numpy as np
import ml_dtypes
from contextlib import ExitStack

import concourse.bass as bass
import concourse.bacc as bacc
import concourse.tile as tile
from concourse import mybir
from concourse.bass_utils import run_bass_kernel_spmd
from concourse.masks import make_identity

B, S, E, R = 64, 2048, 256, 50
NCORES = 8
BC = B // NCORES          # batch per core = 8
NCH = S // 512            # 4 s-chunks of 512
F32 = mybir.dt.float32
BF16 = mybir.dt.bfloat16
F8 = mybir.dt.float8e4
DR = mybir.MatmulPerfMode.DoubleRow
Relu = mybir.ActivationFunctionType.Relu
Tanh = mybir.ActivationFunctionType.Tanh
Exp = mybir.ActivationFunctionType.Exp
Ident = mybir.ActivationFunctionType.Identity
ADD = mybir.AluOpType.add
MAX = mybir.AluOpType.max

# packed bf16 weight blob layout: name -> (col offset, n cols) in [128, WTOT].
# The conv weights live in a separate fp8 blob (w8blob: Kenc hi/lo); the
# row-0 biases live in a 1-row blob (DMAing them as 128-row columns wastes
# 127/128 of the bytes). wblob DMAs in 2 chunks: LSTM block, attention tail.
_WLAYOUT = [("W_ihT", 2048), ("W_hhT", 2048), ("xT", 48), ("h0T", 16),
            ("Wa_mT", 512), ("Wa_qT", 512), ("Went", 4),
            ("Kv_i", 512), ("Kv_f", 512), ("Kv_l", 512), ("W_relT", 2 * R)]
W2END = 2048 + 2048 + 48 + 16
WOFF = {}
_o = 0
for _n, _c in _WLAYOUT:
    WOFF[_n] = (_o, _c)
    _o += _c
WTOT = _o
_BLAYOUT = [("bias_g", 1024), ("b_attn", 256), ("b_conv", 256), ("b_rel", R)]
BOFF = {}
_o = 0
for _n, _c in _BLAYOUT:
    BOFF[_n] = (_o, _c)
    _o += _c
BTOT = _o


def _emit(ctx, tc, nc, io):
    P = 128
    wp = ctx.enter_context(tc.tile_pool(name="wp", bufs=1))
    ep = ctx.enter_context(tc.tile_pool(name="ep", bufs=2))
    bigp = ctx.enter_context(tc.tile_pool(name="bigp", bufs=1))
    stp = ctx.enter_context(tc.tile_pool(name="stp", bufs=18))
    rp = ctx.enter_context(tc.tile_pool(name="rp", bufs=20))
    pcv = ctx.enter_context(tc.tile_pool(name="pcv", bufs=3, space="PSUM"))
    pse = ctx.enter_context(tc.tile_pool(name="pse", bufs=2, space="PSUM"))
    psm = ctx.enter_context(tc.tile_pool(name="psm", bufs=2, space="PSUM"))
    pst = ctx.enter_context(tc.tile_pool(name="pst", bufs=1, space="PSUM"))

    dma = nc.sync.dma_start

    # ---- weights / constants ----
    w8sb = wp.tile([P, 2, 3, 2, 2, P], F8, name="w8blob")
    dma(out=w8sb[:], in_=io["w8blob"].ap())
    K8 = [w8sb[:, 0], w8sb[:, 1]]          # hi/lo: [128, w, ch, half, 128]
    wsb = wp.tile([P, WTOT], BF16, name="wblob")

    def wview(name, *dims):
        o, n = WOFF[name]
        v = wsb[:, o:o + n]
        if not dims:
            return v
        pat = "p (" + " ".join(f"d{i}" for i in range(len(dims) + 1)) + ") -> p " \
            + " ".join(f"d{i}" for i in range(len(dims) + 1))
        return v.rearrange(pat, **{f"d{i}": d for i, d in enumerate(dims)})

    bsb = wp.tile([1, BTOT], BF16, name="bblob")

    def brow(name):
        o, n = BOFF[name]
        return bsb[:, o:o + n]

    W_ihT = wview("W_ihT", 2)          # [128, 2ch, 1024] lhsT e_in -> gates
    W_hhT = wview("W_hhT", 2)
    Wa_mT = wview("Wa_mT", 2)          # [128, 2ch, 256]
    Wa_qT = wview("Wa_qT", 2)
    Kv = [wview("Kv_i", 2), wview("Kv_f", 2), wview("Kv_l", 2)]
    W_relT = wview("W_relT", 2)        # [128, 2ch, 50]
    Went = wview("Went", 2)            # [128, 2ch, 2]
    xT = wview("xT", 3, 2)             # [128, t, ch, BC]
    h0T = wview("h0T", 2)              # [128, ch, BC]
    bias_g = brow("bias_g")
    b_attn = brow("b_attn")
    b_conv = brow("b_conv")
    b_rel = brow("b_rel")

    ones8 = wp.tile([1, BC], BF16, name="ones8")
    nc.vector.memset(ones8[:], 1.0)
    onecol_bf = wp.tile([P, 1], BF16, name="onecol_bf")
    nc.vector.memset(onecol_bf[:], 1.0)
    onerow_bf = wp.tile([1, P], BF16, name="onerow_bf")
    nc.vector.memset(onerow_bf[:], 1.0)
    id_f32 = wp.tile([P, P], F32, name="id_f32")
    make_identity(nc, id_f32[:])

    # state tiles (transposed layout [e-part, ...])
    hQ = wp.tile([P, 2, 3, BC], BF16, name="hQ")           # h1,h2,h3 columns
    hQ8 = [wp.tile([P, 2, 3, BC], F8, name=f"hQ8{i}") for i in range(2)]
    mix_all = wp.tile([P, 3, 2, BC], BF16, name="mix_all")  # normalized mix
    outT = [wp.tile([P, 2, BC], BF16, name=f"outT{a}") for a in range(3)]
    vbT = [wp.tile([P, 3, 2, BC], F32, name=f"vbT{v}") for v in range(2)]
    t1_ps = pst.tile([R, BC], F32, name="t1_ps")

    # ---- encoder DMAs (order chosen so enc8[b] lands before scores/conv(b),
    # encS[b] before mix(b)) ----
    enc8 = [[None] * BC, [None] * BC]   # hi/lo fp8 pairs, [e-part, s] layout
    encS = [None] * BC

    def dma_enc8(b):
        for i, nm in enumerate(("e8hi", "e8lo")):
            t = bigp.tile([P, 2, S], F8, name=f"enc8{nm}{b}")
            dma(out=t[:], in_=io[nm].ap()[b])
            enc8[i][b] = t

    def dma_encS(b):
        t = bigp.tile([P, 16, E], BF16, name=f"encS{b}")
        dma(out=t[:], in_=io["enc_sc"].ap()[b])
        encS[b] = t

    # enc8[0] in halves so conv(b0, j0) can start as early as possible
    for i, nm in enumerate(("e8hi", "e8lo")):
        t0 = bigp.tile([P, 2, S], F8, name=f"enc8{nm}0")
        dma(out=t0[:, :, 0:1024], in_=io[nm].ap()[0][:, :, 0:1024])
        enc8[i][0] = t0
    for i, nm in enumerate(("e8hi", "e8lo")):
        dma(out=enc8[i][0][:, :, 1024:S], in_=io[nm].ap()[0][:, :, 1024:S])
    dma(out=wsb[:, 0:W2END], in_=io["wblob"].ap()[:, 0:W2END])
    dma(out=bsb[:], in_=io["bblob"].ap())
    c0T = wp.tile([P, 2, BC], F32, name="c0T")
    dma(out=c0T[:], in_=io["c0T"].ap())
    dma_enc8(1)
    dma_encS(0)
    dma_encS(1)
    dma_enc8(2)
    dma(out=wsb[:, W2END:], in_=io["wblob"].ap()[:, W2END:])
    dma_encS(2)
    dma_enc8(3)
    dma_encS(3)
    dma_enc8(4)
    dma_encS(4)
    dma_enc8(5)
    dma_encS(5)
    dma_enc8(6)
    dma_enc8(7)
    dma_encS(6)
    dma_encS(7)
    bent64 = wp.tile([64, 1], F32, name="bent64")
    dma(out=bent64[:], in_=io["bent64"].ap())

    out_ap = io["out"].ap()

    # ---- LSTM (batched over BC as matmul free dim) ----
    # NOTE: start=True zeroes the whole 2KB psum bank (lazy), so each psum
    # tile below forms a single accumulation group: start only on its first
    # matmul, stop only on its last; untouched bytes read as zero.
    def gates(t, h_rhs):
        gp = psm.tile([P, 8, BC], F32, name=f"gp{t}", tag="ps")
        for gc in range(8):
            g = gp[:, gc, :]
            sl = slice(gc * 128, (gc + 1) * 128)
            nc.tensor.matmul(g, W_ihT[:, 0, sl], xT[:, t, 0, :],
                             start=(gc == 0), stop=False)
            nc.tensor.matmul(g, W_hhT[:, 0, sl], h_rhs(0), start=False, stop=False)
            nc.tensor.matmul(g, W_ihT[:, 1, sl], xT[:, t, 1, :],
                             start=False, stop=False)
            nc.tensor.matmul(g, W_hhT[:, 1, sl], h_rhs(1), start=False, stop=False)
            nc.tensor.matmul(g, bias_g[:, sl], ones8[:], start=False,
                             stop=(gc == 7))
        return gp

    def lstm_nl(t, gp, c_prev):
        # gate chunks: i=0:2, f=2:4, g=4:6, o=6:8 ; sig(x)=0.5*tanh(x/2)+0.5
        si = ep.tile([P, 2, BC], F32, name=f"si{t}", bufs=1)
        nc.scalar.activation(si[:], gp[:, 0:2, :], Tanh, scale=0.5)
        nc.vector.tensor_scalar(si[:], si[:], 0.5, 0.5,
                                op0=mybir.AluOpType.mult, op1=ADD)
        sf = ep.tile([P, 2, BC], F32, name=f"sf{t}", bufs=1)
        nc.scalar.activation(sf[:], gp[:, 2:4, :], Tanh, scale=0.5)
        nc.vector.tensor_scalar(sf[:], sf[:], 0.5, 0.5,
                                op0=mybir.AluOpType.mult, op1=ADD)
        tg = ep.tile([P, 2, BC], F32, name=f"tg{t}", bufs=1)
        nc.scalar.activation(tg[:], gp[:, 4:6, :], Tanh)
        so = ep.tile([P, 2, BC], F32, name=f"so{t}", bufs=1)
        nc.scalar.activation(so[:], gp[:, 6:8, :], Tanh, scale=0.5)
        nc.vector.tensor_scalar(so[:], so[:], 0.5, 0.5,
                                op0=mybir.AluOpType.mult, op1=ADD)
        c2 = ep.tile([P, 2, BC], F32, name=f"c2_{t}", bufs=1)
        nc.vector.tensor_mul(c2[:], sf[:], c_prev[:])
        tmp = ep.tile([P, 2, BC], F32, name=f"tmp{t}", bufs=1)
        nc.vector.tensor_mul(tmp[:], si[:], tg[:])
        nc.vector.tensor_add(c2[:], c2[:], tmp[:])
        tc2 = ep.tile([P, 2, BC], F32, name=f"tc2_{t}", bufs=1)
        nc.scalar.activation(tc2[:], c2[:], Tanh)
        nc.vector.tensor_mul(hQ[:, :, t, :], so[:], tc2[:])
        # fp8 hi/lo split of h for the scores matmuls
        nc.vector.tensor_copy(hQ8[0][:, :, t, :], hQ[:, :, t, :])
        nc.vector.tensor_sub(hQ8[1][:, :, t, :], hQ[:, :, t, :],
                             hQ8[0][:, :, t, :])
        return c2

    # ---- attention pipeline, per batch (split so conv work can sit between
    # the PE pieces and cover the cross-engine latencies) ----
    def scores_p1(b):
        # scores from the fp8 hi/lo pairs: E.q ~= Eh.qh + Eh.ql + El.qh,
        # each a DoubleRow matmul contracting both e-halves at once
        sc_ps = psm.tile([P, 16, 3], F32, name=f"sc{b}", tag="ps")
        for sc in range(16):
            sl = slice(sc * 128, (sc + 1) * 128)
            for i, (ei, qi) in enumerate(((0, 0), (0, 1), (1, 0))):
                nc.tensor.matmul(sc_ps[:, sc, :], enc8[ei][b][:, :, sl],
                                 hQ8[qi][:, :, :, b],
                                 start=(sc == 0 and i == 0),
                                 stop=(sc == 15 and i == 2), perf_mode=DR)
        # scores are bounded (|s| ~ 40 << 88): unshifted fp32 exp can't overflow
        att = ep.tile([P, 16, 3], BF16, name=f"att{b}", bufs=2)
        nc.scalar.activation(att[:], sc_ps[:], Exp)
        return att

    def scores_p2(b, att):
        sum_ps = psm.tile([1, 16, 3], F32, name=f"sum{b}", tag="ps")
        nc.tensor.matmul(sum_ps[:], onecol_bf[:], att[:], start=True, stop=True)
        s3 = ep.tile([1, 3], F32, name=f"s3_{b}", bufs=2)
        nc.vector.reduce_sum(s3[:], sum_ps.rearrange("p c r -> p r c"),
                             axis=mybir.AxisListType.X)
        rec = ep.tile([1, 3], F32, name=f"rec{b}", bufs=2)
        nc.vector.reciprocal(rec[:], s3[:])
        rsb = ep.tile([P, 3], F32, name=f"rsbs{b}", bufs=2)
        nc.gpsimd.partition_broadcast(rsb[:], rec[:])
        return rsb

    def mix(b, att, rsb_ps):
        mix_ps = psm.tile([P, 2, 3], F32, name=f"mx{b}", tag="ps")
        for half in range(2):
            sl = slice(half * 128, (half + 1) * 128)
            for sc in range(16):
                nc.tensor.matmul(mix_ps[:, half, :], encS[b][:, sc, sl],
                                 att[:, sc, :], start=(half == 0 and sc == 0),
                                 stop=(half == 1 and sc == 15))
        for half in range(2):
            nc.vector.tensor_mul(mix_all[:, :, half, b], mix_ps[:, half, :],
                                 rsb_ps[:])

    def attend_b(a, b, w=1):
        ao = psm.tile([P, 2, w], F32, name=f"ao{a}_{b}", tag="ps")
        for half in range(2):
            o = ao[:, half, :]
            sl = slice(half * 128, (half + 1) * 128)
            for ch in range(2):
                nc.tensor.matmul(o, Wa_mT[:, ch, sl], mix_all[:, a, ch, b:b + w],
                                 start=(half == 0 and ch == 0), stop=False)
                nc.tensor.matmul(o, Wa_qT[:, ch, sl], hQ[:, ch, a, b:b + w],
                                 start=False, stop=False)
            nc.tensor.matmul(o, b_attn[:, sl], ones8[:, 0:w],
                             start=False, stop=(half == 1))
        nc.scalar.activation(outT[a][:, :, b:b + w], ao[:], Tanh)

    def vbias_b(v, b, w=1):
        srcT = outT[v + 1]
        vps = psm.tile([P, 3, 2, w], F32, name=f"vb{v}_{b}", tag="ps")
        for vi in range(3):
            for half in range(2):
                o = vps[:, vi, half, :]
                sl = slice(half * 128, (half + 1) * 128)
                for ch in range(2):
                    nc.tensor.matmul(o, Kv[vi][:, ch, sl], srcT[:, ch, b:b + w],
                                     start=(vi == 0 and half == 0 and ch == 0),
                                     stop=False)
                nc.tensor.matmul(o, b_conv[:, sl], ones8[:, 0:w],
                                 start=False, stop=(vi == 2 and half == 1))
        nc.scalar.copy(vbT[v][:, :, :, b:b + w], vps[:])

    def t1_col(b, w=1):
        o = t1_ps[:, b:b + w]
        for ch in range(2):
            nc.tensor.matmul(o, W_relT[:, ch, :], outT[0][:, ch, b:b + w],
                             start=(b == 0 and ch == 0), stop=False)
        nc.tensor.matmul(o, b_rel[:], ones8[:, 0:w], start=False,
                         stop=(b + w == BC))

    # ---- conv (3-tap over enc; fp8 hi/lo split: K.e ~= Kh.eh + Kh.el +
    # Kl.eh, DoubleRow contracting both e_in halves per matmul) ----
    def conv_half(b, j, half):
        s0 = j * 512
        ps = pcv.tile([P, 512], F32, name="conv_ps")
        first = True
        for w in (1, 0, 2):
            lo = s0 + w - 1
            ob, oe = 0, 512
            if lo < 0:
                ob, lo = 1, 0
            elif lo + 512 > S:
                oe = 511
            for ki, ei in ((0, 0), (0, 1), (1, 0)):
                nc.tensor.matmul(ps[:, ob:oe], K8[ki][:, w, :, half, :],
                                 enc8[ei][b][:, :, lo:lo + (oe - ob)],
                                 start=first, stop=(w == 2 and ki == 1),
                                 perf_mode=DR)
                first = False
        st = stp.tile([P, 512], BF16, name="cvst")
        # alternate the psum->sbuf staging between Activation and DVE so
        # neither queue's head-of-line blocking can stall the conv psum pool
        # (GPSIMD cannot read PSUM on hardware)
        nc.scalar.copy(st[:], ps[:])
        return st

    eps = [None] * BC
    stages = [[None, None] for _ in range(NCH)]  # stages of batch currently conv'd
    stage_bufs = {}

    def relus_j(b, j, sts):
        # relu(conv + vbias) for both heads/halves; emitted as early as its
        # inputs allow so the DVE never gates the entity-head matmuls
        rs = {}
        for half in range(2):       # half-major: half-1 relus never block
            for v in range(2):      # a half-0 consumer in the DVE queue
                r = rp.tile([P, 512], BF16, name="relu")
                nc.vector.tensor_scalar(r[:], sts[half][:],
                                        vbT[v][:, 0, half, b:b + 1], 0.0,
                                        op0=ADD, op1=MAX)
                if j == 0:
                    nc.vector.tensor_scalar(r[:, 0:1], sts[half][:, 0:1],
                                            vbT[v][:, 1, half, b:b + 1], 0.0,
                                            op0=ADD, op1=MAX)
                if j == NCH - 1:
                    nc.vector.tensor_scalar(r[:, 511:512], sts[half][:, 511:512],
                                            vbT[v][:, 2, half, b:b + 1], 0.0,
                                            op0=ADD, op1=MAX)
                rs[v * 2 + half] = r
        return rs

    def entmm_j(b, j, rs):
        for half in range(2):
            for v in range(2):
                r = rs[v * 2 + half]
                for sc4 in range(4):
                    c = (j * 4 + sc4) * 4 + v * 2
                    nc.tensor.matmul(eps[b][:, c:c + 2],
                                     r[:, sc4 * 128:(sc4 + 1) * 128],
                                     Went[:, half, :],
                                     start=(j == 0 and v == 0 and half == 0
                                            and sc4 == 0),
                                     stop=(j == NCH - 1 and v == 1 and half == 1
                                           and sc4 == 3))

    def ent_j(b, j, sts):
        entmm_j(b, j, relus_j(b, j, sts))

    def ent_flush(b, part=None):
        # eps[b] [128 s, 64 (sc,v,e)] -> transpose -> +bias -> one DMA.
        # part splits the flush in column halves so the tail can overlap.
        lo, n = (0, 64) if part is None else (part * 32, 32)
        esb = ep.tile([P, n], F32, name=f"esb{b}_{part}", bufs=1)
        nc.scalar.copy(esb[:], eps[b][:, lo:lo + n])
        trp = psm.tile([n, P], F32, name=f"trp{b}_{part}", tag="ps")
        nc.tensor.transpose(trp[:], esb[:], id_f32[:])
        trow = ep.tile([n, P], F32, name=f"trow{b}_{part}", bufs=1)
        nc.scalar.activation(trow[:], trp[:], Ident, bias=bent64[lo:lo + n, :])
        ov = out_ap[b:b + 1, R:R + 4 * S].rearrange(
            "o (k c p) -> o c k p", k=4, c=16, p=128)
        dma(out=ov[:, lo // 4:(lo + n) // 4], in_=trow[:])

    def chain(b):
      with tc.high_priority(400):
        eps[b] = pse.tile([P, 64], F32, name=f"eps{b}", tag="eps")
        att = scores_p1(b)
        rsb = scores_p2(b, att)
        mix(b, att, rsb)
        for a in range(3):
            attend_b(a, b)
        t1_col(b)
        vbias_b(0, b)
        vbias_b(1, b)

    def batch_block(b, chain_self=True, chain_next=False):
        """scores/mix/attends/vb interleaved into conv(b) so the PE reaches
        each piece roughly when its DMA dependency lands and the cross-engine
        latencies hide behind conv matmuls."""
        if chain_self:
            eps[b] = pse.tile([P, 64], F32, name=f"eps{b}", tag="eps")
            att = scores_p1(b)
        rsA = [relus_j(b - 1, j, stage_bufs[(b - 1, j)]) for j in (0, 1)]
        stages[0] = [conv_half(b, 0, h) for h in range(2)]
        if chain_self:
            rsb = scores_p2(b, att)
        rsB = [relus_j(b - 1, j, stage_bufs[(b - 1, j)]) for j in (2, 3)]
        stages[1] = [conv_half(b, 1, h) for h in range(2)]
        if chain_self:
            mix(b, att, rsb)
        stages[2] = [conv_half(b, 2, h) for h in range(2)]
        if chain_self:
            for a in range(3):
                attend_b(a, b)
            t1_col(b)
            vbias_b(0, b)
            vbias_b(1, b)
        entmm_j(b - 1, 0, rsA[0])
        entmm_j(b - 1, 1, rsA[1])
        if chain_next:
            bn = b + 1
            eps[bn] = pse.tile([P, 64], F32, name=f"eps{bn}", tag="eps")
            attN = scores_p1(bn)
        stages[3] = [conv_half(b, 3, h) for h in range(2)]
        if chain_next:
            rsbN = scores_p2(bn, attN)
        entmm_j(b - 1, 2, rsB[0])
        entmm_j(b - 1, 3, rsB[1])
        if chain_next:
            mix(bn, attN, rsbN)
        ent_flush(b - 1)
        if chain_next:
            for a in range(3):
                attend_b(a, bn)
            t1_col(bn)
            vbias_b(0, bn)
            vbias_b(1, bn)
        for j in range(NCH):
            del stage_bufs[(b - 1, j)]
            stage_bufs[(b, j)] = stages[j]

    def block6(b=BC - 2):
        """penultimate batch: both remaining attention chains are emitted
        before any of this block's conv staging, so no conv copy ever queues
        behind a chain activation; entity heads then chase the conv."""
        b7 = b + 1
        eps[b7] = pse.tile([P, 64], F32, name=f"eps{b7}", tag="eps")
        att7 = scores_p1(b7)
        rsA = [relus_j(b - 1, j, stage_bufs[(b - 1, j)]) for j in (0, 1)]
        stages[0] = [conv_half(b, 0, h) for h in range(2)]
        rsb7 = scores_p2(b7, att7)
        rsB = [relus_j(b - 1, j, stage_bufs[(b - 1, j)]) for j in (2, 3)]
        stages[1] = [conv_half(b, 1, h) for h in range(2)]
        mix(b7, att7, rsb7)
        for a in range(3):
            attend_b(a, b7)
        t1_col(b7)
        vbias_b(0, b7)
        vbias_b(1, b7)
        t1_flush()
        entmm_j(b - 1, 0, rsA[0])
        entmm_j(b - 1, 1, rsA[1])
        r60 = relus_j(b, 0, stages[0])
        stages[2] = [conv_half(b, 2, h) for h in range(2)]
        entmm_j(b - 1, 2, rsB[0])
        entmm_j(b - 1, 3, rsB[1])
        ent_flush(b - 1)
        r61 = relus_j(b, 1, stages[1])
        entmm_j(b, 0, r60)
        stages[3] = [conv_half(b, 3, h) for h in range(2)]
        entmm_j(b, 1, r61)
        r62 = relus_j(b, 2, stages[2])
        s70 = [conv_half(b7, 0, h) for h in range(2)]
        entmm_j(b, 2, r62)
        r63 = relus_j(b, 3, stages[3])
        s71 = [conv_half(b7, 1, h) for h in range(2)]
        entmm_j(b, 3, r63)
        ent_flush(b)
        r70 = relus_j(b7, 0, s70)
        s72 = [conv_half(b7, 2, h) for h in range(2)]
        entmm_j(b7, 0, r70)
        r71 = relus_j(b7, 1, s71)
        s73 = [conv_half(b7, 3, h) for h in range(2)]
        entmm_j(b7, 1, r71)
        r72 = relus_j(b7, 2, s72)
        entmm_j(b7, 2, r72)
        r73 = relus_j(b7, 3, s73)
        entmm_j(b7, 3, r73)
        ent_flush(b7)

    # ---- emission: PE p-state warmup (tiny matmuls on memset constants, no
    # DMA deps) so the conv runs at full clock from its first matmul ----
    for wi in range(30):
        wps = psm.tile([BC, P], F32, name=f"warm{wi}", tag="ps")
        nc.tensor.matmul(wps[:], ones8[:], onerow_bf[:], start=True, stop=True)

    # ---- conv(b0) interleaved with the LSTM chain ----
    eps[0] = pse.tile([P, 64], F32, name="eps0", tag="eps")
    stages[0] = [conv_half(0, 0, h) for h in range(2)]
    gp = gates(0, lambda ch: h0T[:, ch, :])
    c1 = lstm_nl(0, gp, c0T)
    stages[1] = [conv_half(0, 1, h) for h in range(2)]
    gp = gates(1, lambda ch: hQ[:, ch, 0, :])
    c2 = lstm_nl(1, gp, c1)
    stages[2] = [conv_half(0, 2, h) for h in range(2)]
    gp = gates(2, lambda ch: hQ[:, ch, 1, :])
    lstm_nl(2, gp, c2)
    stages[3] = [conv_half(0, 3, h) for h in range(2)]
    att0 = scores_p1(0)
    rsb0 = scores_p2(0, att0)
    mix(0, att0, rsb0)
    for a in range(3):
        attend_b(a, 0)
    t1_col(0)
    vbias_b(0, 0)
    vbias_b(1, 0)
    for j in range(NCH):
        stage_bufs[(0, j)] = stages[j]

    def t1_flush():
        t1sb = ep.tile([R, BC], F32, name="t1sb")
        nc.scalar.copy(t1sb[:], t1_ps[:])
        t1tr = psm.tile([BC, R], F32, name="t1tr", tag="ps")
        nc.tensor.transpose(t1tr[:], t1sb[:], id_f32[:R, :R])
        t1row = ep.tile([BC, R], F32, name="t1row")
        nc.scalar.copy(t1row[:], t1tr[:])
        dma(out=out_ap[:, 0:R], in_=t1row[:])

    batch_block(1, chain_self=True, chain_next=True)
    batch_block(2, chain_self=False, chain_next=True)
    batch_block(3, chain_self=False, chain_next=True)
    batch_block(4, chain_self=False, chain_next=True)
    batch_block(5, chain_self=False, chain_next=True)
    block6()


def build_nc():
    nc = bacc.Bacc("TRN2", target_bir_lowering=False, debug=False)
    io = {}

    def din(name, shape, dt):
        io[name] = nc.dram_tensor(name, shape, dt, kind="ExternalInput")

    din("e8hi", [BC, 128, 2, S], F8)
    din("e8lo", [BC, 128, 2, S], F8)
    din("enc_sc", [BC, 128, 16, E], BF16)
    din("wblob", [128, WTOT], BF16)
    din("bblob", [1, BTOT], BF16)
    din("w8blob", [128, 2, 3, 2, 2, 128], F8)
    din("bent64", [64, 1], F32)
    din("c0T", [128, 2, BC], F32)
    io["out"] = nc.dram_tensor("out", [BC, R + 4 * S], F32, kind="ExternalOutput")

    with ExitStack() as ctx:
        t = ctx.enter_context(tile.TileContext(nc))
        _emit(ctx, t, nc, io)
    nc.compile()
    return nc


def _pack2(w):  # [256, N] fp32 -> [128, 2, N]
    return np.ascontiguousarray(w.reshape(2, 128, -1).transpose(1, 0, 2))


def prepare_in_maps(inputs):
    bf = ml_dtypes.bfloat16
    f8 = ml_dtypes.float8_e4m3
    enc = np.asarray(inputs["encoder_o"], np.float32)
    enc_bf = enc.astype(bf)
    # [b, p, ch, s] layout: x[b, p, ch, s] = v[b, s, ch*128+p]
    def to_cs(v):
        return np.ascontiguousarray(
            v.transpose(0, 2, 1).reshape(B, 2, 128, S).transpose(0, 2, 1, 3))
    enc_hi = enc.astype(f8)
    enc_lo = (enc - enc_hi.astype(np.float32)).astype(f8)
    e8hi = to_cs(enc_hi)
    e8lo = to_cs(enc_lo)
    W_ih = np.asarray(inputs["W_ih"], np.float32)
    W_hh = np.asarray(inputs["W_hh"], np.float32)
    W_attn = np.asarray(inputs["W_attn"], np.float32)
    kern = np.asarray(inputs["W_conv"], np.float32).transpose(2, 1, 0)  # [3,2E,E]
    Kenc_ = kern[:, :E, :]
    Kv = kern[:, E:, :]
    Kv_i, Kv_f, Kv_l = Kv.sum(0), Kv[1] + Kv[2], Kv[0] + Kv[1]
    # Kenc fp8 hi/lo pack [128, 2, 3, 2, 2, 128]:
    # [p,i,w,ch,half,m] = Khi/lo[w, ch*128+p, half*128+m]
    K_hi = Kenc_.astype(f8)
    K_lo = (Kenc_ - K_hi.astype(np.float32)).astype(f8)
    kp = np.stack([
        k.reshape(3, 2, 128, 2, 128).transpose(2, 0, 1, 3, 4)
        for k in (K_hi, K_lo)], 1)  # [128, 2, 3, 2, 2, 128]
    We = np.stack([np.asarray(inputs["W_ent1"])[0], np.asarray(inputs["W_ent2"])[0]], 1)
    x1 = np.broadcast_to(np.asarray(inputs["sos_emb"])[0], (B, E))
    x2 = np.asarray(inputs["rel_emb"])[np.asarray(inputs["r_in"]).astype(np.int64)]
    idx = np.arange(B)
    k1 = np.asarray(inputs["k1"])[:, 0].astype(np.int64)
    k2 = np.asarray(inputs["k2"])[:, 0].astype(np.int64)
    x3 = enc[idx, k1] + enc[idx, k2]
    X = np.stack([x1, x2, x3], 0).astype(np.float32)      # [3,B,E]
    h0 = np.asarray(inputs["h0"], np.float32)[0]
    c0 = np.asarray(inputs["c0"], np.float32)
    c0 = c0[0] if c0.ndim == 3 else c0                    # [B, E]

    wsh = np.zeros((128, WTOT), np.float32)
    bsh = np.zeros((1, BTOT), np.float32)

    def put(name, arr):                      # arr -> [128, n] block
        o, n = WOFF[name]
        wsh[:, o:o + n] = arr.reshape(128, n)

    def putrow(name, vec):                   # 1-row bias blob entries
        o, n = BOFF[name]
        bsh[0, o:o + n] = vec.ravel()

    put("W_ihT", _pack2(W_ih.T))
    put("W_hhT", _pack2(W_hh.T))
    put("Wa_mT", _pack2(W_attn[:, :E].T))
    put("Wa_qT", _pack2(W_attn[:, E:].T))
    put("Kv_i", _pack2(Kv_i))
    put("Kv_f", _pack2(Kv_f))
    put("Kv_l", _pack2(Kv_l))
    put("W_relT", _pack2(np.asarray(inputs["W_rel"], np.float32).T))
    put("Went", _pack2(We))
    putrow("bias_g", np.asarray(inputs["b_ih"], np.float32)
           + np.asarray(inputs["b_hh"], np.float32))
    putrow("b_attn", np.asarray(inputs["b_attn"], np.float32))
    putrow("b_conv", np.asarray(inputs["b_conv"], np.float32))
    putrow("b_rel", np.asarray(inputs["b_rel"], np.float32))
    be1 = float(np.asarray(inputs["b_ent1"]).ravel()[0])
    be2 = float(np.asarray(inputs["b_ent2"]).ravel()[0])
    bent64 = np.ascontiguousarray(
        np.tile(np.array([be1, be2], np.float32), 32).reshape(64, 1))
    in_maps = []
    for c in range(NCORES):
        sl = slice(c * BC, (c + 1) * BC)
        w = wsh.copy()
        xs = X[:, sl]                                      # [3,BC,E]
        xo, xn = WOFF["xT"]
        w[:, xo:xo + xn] = xs.transpose(2, 0, 1).reshape(
            2, 128, 3, BC).transpose(1, 2, 0, 3).reshape(128, xn)
        ho, hn = WOFF["h0T"]
        w[:, ho:ho + hn] = h0[sl].T.reshape(2, 128, BC).transpose(
            1, 0, 2).reshape(128, hn)
        m = {
            "e8hi": np.ascontiguousarray(e8hi[sl]),
            "e8lo": np.ascontiguousarray(e8lo[sl]),
            "enc_sc": np.ascontiguousarray(
                enc_bf[sl].reshape(BC, 16, 128, E).transpose(0, 2, 1, 3)),
            "wblob": w.astype(bf),
            "bblob": bsh.astype(bf),
            "w8blob": np.ascontiguousarray(kp),
            "bent64": bent64,
            "c0T": np.ascontiguousarray(
                c0[sl].T.reshape(2, 128, BC).transpose(1, 0, 2)),
        }
        in_maps.append(m)
    return in_maps


_NC_CACHE = {}


def get_nc():
    if "nc" not in _NC_CACHE:
        _NC_CACHE["nc"] = build_nc()
    return _NC_CACHE["nc"]


def kernel(**inputs) -> np.ndarray:
    nc = get_nc()
    in_maps = prepare_in_maps(inputs)
    res = run_bass_kernel_spmd(nc, in_maps, core_ids=list(range(NCORES)))
    return np.concatenate([r["out"] for r in res.results], 0).astype(np.float32)


if __name__ == "__main__":
    import jax
    import reference as refmod
    with jax.default_device(jax.devices("cpu")[0]):
        inputs = {k: np.asarray(v) for k, v in refmod.setup_inputs().items()}
        expected = np.asarray(refmod.reference(**inputs))
    actual = kernel(**inputs)
    err = np.abs(actual - expected)
    print("max abs err:", err.max(), "rel:", err.max() / np.abs(expected).max())


# revision 80
# speedup vs baseline: 1.0025x; 1.0025x over previous
"""Trainium2 Bass kernel for nn_Decoder (3-step LSTM decoder w/ Luong attention
+ conv1d entity heads). Data-parallel over batch: B=64 -> 8 cores x 8.

Restructured so every non-conv matmul keeps its large dims on the PE
partition/stationary side and streams only a tiny output free dim (the PE
cost is out_free_size cycles): LSTM gates / scores / mix / attends / vbias /
relation logits all produce [*, batch<=8] or [*, 3] outputs; the entity-head
reduction consumes each relu tile as the stationary operand against
Went [128, 2] (2-cycle matmuls) and the per-batch result is PE-transposed
once and written with a single DMA per batch.

Decomposition (validated vs reference to 5e-7):
  - conv1d over feat=[enc, broadcast(o)] splits into a 3-tap matmul conv over
    enc (shared by both ent_heads calls) plus a per-batch bias vec (with
    first/last-column variants for the SAME-padding edges).
  - attend(q) = tanh(mix @ Wa[:, :E].T + q @ Wa[:, E:].T + b) with
    mix = softmax(q.enc) @ enc.
All heavy matmuls run in bf16 (fp32 PSUM accumulation).
"""
import numpy as np
import ml_dtypes
from contextlib import ExitStack

import concourse.bass as bass
import concourse.bacc as bacc
import concourse.tile as tile
from concourse import mybir
from concourse.bass_utils import run_bass_kernel_spmd
from concourse.masks import make_identity

B, S, E, R = 64, 2048, 256, 50
NCORES = 8
BC = B // NCORES          # batch per core = 8
NCH = S // 512            # 4 s-chunks of 512
F32 = mybir.dt.float32
BF16 = mybir.dt.bfloat16
F8 = mybir.dt.float8e4
DR = mybir.MatmulPerfMode.DoubleRow
Relu = mybir.ActivationFunctionType.Relu
Tanh = mybir.ActivationFunctionType.Tanh
Exp = mybir.ActivationFunctionType.Exp
Ident = mybir.ActivationFunctionType.Identity
ADD = mybir.AluOpType.add
MAX = mybir.AluOpType.max

# packed bf16 weight blob layout: name -> (col offset, n cols) in [128, WTOT].
# The conv weights live in a separate fp8 blob (w8blob: Kenc hi/lo); the
# row-0 biases live in a 1-row blob (DMAing them as 128-row columns wastes
# 127/128 of the bytes). wblob DMAs in 2 chunks: LSTM block, attention tail.
_WLAYOUT = [("W_ihT", 2048), ("W_hhT", 2048), ("xT", 48), ("h0T", 16),
            ("Wa_mT", 512), ("Wa_qT", 512), ("Went", 4),
            ("Kv_i", 512), ("Kv_f", 512), ("Kv_l", 512), ("W_relT", 2 * R)]
W2END = 2048 + 2048 + 48 + 16
WOFF = {}
_o = 0
for _n, _c in _WLAYOUT:
    WOFF[_n] = (_o, _c)
    _o += _c
WTOT = _o
_BLAYOUT = [("bias_g", 1024), ("b_attn", 256), ("b_conv", 256), ("b_rel", R)]
BOFF = {}
_o = 0
for _n, _c in _BLAYOUT:
    BOFF[_n] = (_o, _c)
    _o += _c
BTOT = _o


def _emit(ctx, tc, nc, io):
    P = 128
    wp = ctx.enter_context(tc.tile_pool(name="wp", bufs=1))
    ep = ctx.enter_context(tc.tile_pool(name="ep", bufs=2))
    bigp = ctx.enter_context(tc.tile_pool(name="bigp", bufs=1))
    stp = ctx.enter_context(tc.tile_pool(name="stp", bufs=19))
    rp = ctx.enter_context(tc.tile_pool(name="rp", bufs=20))
    pcv = ctx.enter_context(tc.tile_pool(name="pcv", bufs=3, space="PSUM"))
    pse = ctx.enter_context(tc.tile_pool(name="pse", bufs=2, space="PSUM"))
    psm = ctx.enter_context(tc.tile_pool(name="psm", bufs=2, space="PSUM"))
    pst = ctx.enter_context(tc.tile_pool(name="pst", bufs=1, space="PSUM"))

    dma = nc.sync.dma_start

    # ---- weights / constants ----
    w8sb = wp.tile([P, 2, 3, 2, 2, P], F8, name="w8blob")
    dma(out=w8sb[:], in_=io["w8blob"].ap())
    K8 = [w8sb[:, 0], w8sb[:, 1]]          # hi/lo: [128, w, ch, half, 128]
    wsb = wp.tile([P, WTOT], BF16, name="wblob")

    def wview(name, *dims):
        o, n = WOFF[name]
        v = wsb[:, o:o + n]
        if not dims:
            return v
        pat = "p (" + " ".join(f"d{i}" for i in range(len(dims) + 1)) + ") -> p " \
            + " ".join(f"d{i}" for i in range(len(dims) + 1))
        return v.rearrange(pat, **{f"d{i}": d for i, d in enumerate(dims)})

    bsb = wp.tile([1, BTOT], BF16, name="bblob")

    def brow(name):
        o, n = BOFF[name]
        return bsb[:, o:o + n]

    W_ihT = wview("W_ihT", 2)          # [128, 2ch, 1024] lhsT e_in -> gates
    W_hhT = wview("W_hhT", 2)
    Wa_mT = wview("Wa_mT", 2)          # [128, 2ch, 256]
    Wa_qT = wview("Wa_qT", 2)
    Kv = [wview("Kv_i", 2), wview("Kv_f", 2), wview("Kv_l", 2)]
    W_relT = wview("W_relT", 2)        # [128, 2ch, 50]
    Went = wview("Went", 2)            # [128, 2ch, 2]
    xT = wview("xT", 3, 2)             # [128, t, ch, BC]
    h0T = wview("h0T", 2)              # [128, ch, BC]
    bias_g = brow("bias_g")
    b_attn = brow("b_attn")
    b_conv = brow("b_conv")
    b_rel = brow("b_rel")

    ones8 = wp.tile([1, BC], BF16, name="ones8")
    nc.vector.memset(ones8[:], 1.0)
    onecol_bf = wp.tile([P, 1], BF16, name="onecol_bf")
    nc.vector.memset(onecol_bf[:], 1.0)
    onerow_bf = wp.tile([1, P], BF16, name="onerow_bf")
    nc.vector.memset(onerow_bf[:], 1.0)
    id_f32 = wp.tile([P, P], F32, name="id_f32")
    make_identity(nc, id_f32[:])

    # state tiles (transposed layout [e-part, ...])
    hQ = wp.tile([P, 2, 3, BC], BF16, name="hQ")           # h1,h2,h3 columns
    hQ8 = [wp.tile([P, 2, 3, BC], F8, name=f"hQ8{i}") for i in range(2)]
    mix_all = wp.tile([P, 3, 2, BC], BF16, name="mix_all")  # normalized mix
    outT = [wp.tile([P, 2, BC], BF16, name=f"outT{a}") for a in range(3)]
    vbT = [wp.tile([P, 3, 2, BC], F32, name=f"vbT{v}") for v in range(2)]
    t1_ps = pst.tile([R, BC], F32, name="t1_ps")

    # ---- encoder DMAs (order chosen so enc8[b] lands before scores/conv(b),
    # encS[b] before mix(b)) ----
    enc8 = [[None] * BC, [None] * BC]   # hi/lo fp8 pairs, [e-part, s] layout
    encS = [None] * BC

    def dma_enc8(b):
        for i, nm in enumerate(("e8hi", "e8lo")):
            t = bigp.tile([P, 2, S], F8, name=f"enc8{nm}{b}")
            dma(out=t[:], in_=io[nm].ap()[b])
            enc8[i][b] = t

    def dma_encS(b):
        t = bigp.tile([P, 16, E], BF16, name=f"encS{b}")
        dma(out=t[:], in_=io["enc_sc"].ap()[b])
        encS[b] = t

    # enc8[0] in halves so conv(b0, j0) can start as early as possible
    for i, nm in enumerate(("e8hi", "e8lo")):
        t0 = bigp.tile([P, 2, S], F8, name=f"enc8{nm}0")
        dma(out=t0[:, :, 0:1024], in_=io[nm].ap()[0][:, :, 0:1024])
        enc8[i][0] = t0
    for i, nm in enumerate(("e8hi", "e8lo")):
        dma(out=enc8[i][0][:, :, 1024:S], in_=io[nm].ap()[0][:, :, 1024:S])
    dma(out=wsb[:, 0:W2END], in_=io["wblob"].ap()[:, 0:W2END])
    dma(out=bsb[:], in_=io["bblob"].ap())
    c0T = wp.tile([P, 2, BC], F32, name="c0T")
    dma(out=c0T[:], in_=io["c0T"].ap())
    dma_enc8(1)
    dma_encS(0)
    dma_encS(1)
    dma_enc8(2)
    dma(out=wsb[:, W2END:], in_=io["wblob"].ap()[:, W2END:])
    dma_encS(2)
    dma_enc8(3)
    dma_encS(3)
    dma_enc8(4)
    dma_encS(4)
    dma_enc8(5)
    dma_encS(5)
    dma_enc8(6)
    dma_enc8(7)
    dma_encS(6)
    dma_encS(7)
    bent64 = wp.tile([64, 1], F32, name="bent64")
    dma(out=bent64[:], in_=io["bent64"].ap())

    out_ap = io["out"].ap()

    # ---- LSTM (batched over BC as matmul free dim) ----
    # NOTE: start=True zeroes the whole 2KB psum bank (lazy), so each psum
    # tile below forms a single accumulation group: start only on its first
    # matmul, stop only on its last; untouched bytes read as zero.
    def gates(t, h_rhs):
        gp = psm.tile([P, 8, BC], F32, name=f"gp{t}", tag="ps")
        for gc in range(8):
            g = gp[:, gc, :]
            sl = slice(gc * 128, (gc + 1) * 128)
            nc.tensor.matmul(g, W_ihT[:, 0, sl], xT[:, t, 0, :],
                             start=(gc == 0), stop=False)
            nc.tensor.matmul(g, W_hhT[:, 0, sl], h_rhs(0), start=False, stop=False)
            nc.tensor.matmul(g, W_ihT[:, 1, sl], xT[:, t, 1, :],
                             start=False, stop=False)
            nc.tensor.matmul(g, W_hhT[:, 1, sl], h_rhs(1), start=False, stop=False)
            nc.tensor.matmul(g, bias_g[:, sl], ones8[:], start=False,
                             stop=(gc == 7))
        return gp

    def lstm_nl(t, gp, c_prev):
        # gate chunks: i=0:2, f=2:4, g=4:6, o=6:8 ; sig(x)=0.5*tanh(x/2)+0.5
        si = ep.tile([P, 2, BC], F32, name=f"si{t}", bufs=1)
        nc.scalar.activation(si[:], gp[:, 0:2, :], Tanh, scale=0.5)
        nc.vector.tensor_scalar(si[:], si[:], 0.5, 0.5,
                                op0=mybir.AluOpType.mult, op1=ADD)
        sf = ep.tile([P, 2, BC], F32, name=f"sf{t}", bufs=1)
        nc.scalar.activation(sf[:], gp[:, 2:4, :], Tanh, scale=0.5)
        nc.vector.tensor_scalar(sf[:], sf[:], 0.5, 0.5,
                                op0=mybir.AluOpType.mult, op1=ADD)
        tg = ep.tile([P, 2, BC], F32, name=f"tg{t}", bufs=1)
        nc.scalar.activation(tg[:], gp[:, 4:6, :], Tanh)
        so = ep.tile([P, 2, BC], F32, name=f"so{t}", bufs=1)
        nc.scalar.activation(so[:], gp[:, 6:8, :], Tanh, scale=0.5)
        nc.vector.tensor_scalar(so[:], so[:], 0.5, 0.5,
                                op0=mybir.AluOpType.mult, op1=ADD)
        c2 = ep.tile([P, 2, BC], F32, name=f"c2_{t}", bufs=1)
        nc.vector.tensor_mul(c2[:], sf[:], c_prev[:])
        tmp = ep.tile([P, 2, BC], F32, name=f"tmp{t}", bufs=1)
        nc.vector.tensor_mul(tmp[:], si[:], tg[:])
        nc.vector.tensor_add(c2[:], c2[:], tmp[:])
        tc2 = ep.tile([P, 2, BC], F32, name=f"tc2_{t}", bufs=1)
        nc.scalar.activation(tc2[:], c2[:], Tanh)
        nc.vector.tensor_mul(hQ[:, :, t, :], so[:], tc2[:])
        # fp8 hi/lo split of h for the scores matmuls
        nc.vector.tensor_copy(hQ8[0][:, :, t, :], hQ[:, :, t, :])
        nc.vector.tensor_sub(hQ8[1][:, :, t, :], hQ[:, :, t, :],
                             hQ8[0][:, :, t, :])
        return c2

    # ---- attention pipeline, per batch (split so conv work can sit between
    # the PE pieces and cover the cross-engine latencies) ----
    def scores_p1(b):
        # scores from the fp8 hi/lo pairs: E.q ~= Eh.qh + Eh.ql + El.qh,
        # each a DoubleRow matmul contracting both e-halves at once
        sc_ps = psm.tile([P, 16, 3], F32, name=f"sc{b}", tag="ps")
        for sc in range(16):
            sl = slice(sc * 128, (sc + 1) * 128)
            for i, (ei, qi) in enumerate(((0, 0), (0, 1), (1, 0))):
                nc.tensor.matmul(sc_ps[:, sc, :], enc8[ei][b][:, :, sl],
                                 hQ8[qi][:, :, :, b],
                                 start=(sc == 0 and i == 0),
                                 stop=(sc == 15 and i == 2), perf_mode=DR)
        # scores are bounded (|s| ~ 40 << 88): unshifted fp32 exp can't overflow
        att = ep.tile([P, 16, 3], BF16, name=f"att{b}", bufs=2)
        nc.scalar.activation(att[:], sc_ps[:], Exp)
        return att

    def scores_p2(b, att):
        sum_ps = psm.tile([1, 16, 3], F32, name=f"sum{b}", tag="ps")
        nc.tensor.matmul(sum_ps[:], onecol_bf[:], att[:], start=True, stop=True)
        s3 = ep.tile([1, 3], F32, name=f"s3_{b}", bufs=2)
        nc.vector.reduce_sum(s3[:], sum_ps.rearrange("p c r -> p r c"),
                             axis=mybir.AxisListType.X)
        rec = ep.tile([1, 3], F32, name=f"rec{b}", bufs=2)
        nc.vector.reciprocal(rec[:], s3[:])
        rsb = ep.tile([P, 3], F32, name=f"rsbs{b}", bufs=2)
        nc.gpsimd.partition_broadcast(rsb[:], rec[:])
        return rsb

    def mix(b, att, rsb_ps):
        mix_ps = psm.tile([P, 2, 3], F32, name=f"mx{b}", tag="ps")
        for half in range(2):
            sl = slice(half * 128, (half + 1) * 128)
            for sc in range(16):
                nc.tensor.matmul(mix_ps[:, half, :], encS[b][:, sc, sl],
                                 att[:, sc, :], start=(half == 0 and sc == 0),
                                 stop=(half == 1 and sc == 15))
        for half in range(2):
            nc.vector.tensor_mul(mix_all[:, :, half, b], mix_ps[:, half, :],
                                 rsb_ps[:])

    def attend_b(a, b, w=1):
        ao = psm.tile([P, 2, w], F32, name=f"ao{a}_{b}", tag="ps")
        for half in range(2):
            o = ao[:, half, :]
            sl = slice(half * 128, (half + 1) * 128)
            for ch in range(2):
                nc.tensor.matmul(o, Wa_mT[:, ch, sl], mix_all[:, a, ch, b:b + w],
                                 start=(half == 0 and ch == 0), stop=False)
                nc.tensor.matmul(o, Wa_qT[:, ch, sl], hQ[:, ch, a, b:b + w],
                                 start=False, stop=False)
            nc.tensor.matmul(o, b_attn[:, sl], ones8[:, 0:w],
                             start=False, stop=(half == 1))
        nc.scalar.activation(outT[a][:, :, b:b + w], ao[:], Tanh)

    def vbias_b(v, b, w=1):
        srcT = outT[v + 1]
        vps = psm.tile([P, 3, 2, w], F32, name=f"vb{v}_{b}", tag="ps")
        for vi in range(3):
            for half in range(2):
                o = vps[:, vi, half, :]
                sl = slice(half * 128, (half + 1) * 128)
                for ch in range(2):
                    nc.tensor.matmul(o, Kv[vi][:, ch, sl], srcT[:, ch, b:b + w],
                                     start=(vi == 0 and half == 0 and ch == 0),
                                     stop=False)
                nc.tensor.matmul(o, b_conv[:, sl], ones8[:, 0:w],
                                 start=False, stop=(vi == 2 and half == 1))
        nc.scalar.copy(vbT[v][:, :, :, b:b + w], vps[:])

    def t1_col(b, w=1):
        o = t1_ps[:, b:b + w]
        for ch in range(2):
            nc.tensor.matmul(o, W_relT[:, ch, :], outT[0][:, ch, b:b + w],
                             start=(b == 0 and ch == 0), stop=False)
        nc.tensor.matmul(o, b_rel[:], ones8[:, 0:w], start=False,
                         stop=(b + w == BC))

    # ---- conv (3-tap over enc; fp8 hi/lo split: K.e ~= Kh.eh + Kh.el +
    # Kl.eh, DoubleRow contracting both e_in halves per matmul) ----
    def conv_half(b, j, half):
        s0 = j * 512
        ps = pcv.tile([P, 512], F32, name="conv_ps")
        first = True
        for w in (1, 0, 2):
            lo = s0 + w - 1
            ob, oe = 0, 512
            if lo < 0:
                ob, lo = 1, 0
            elif lo + 512 > S:
                oe = 511
            for ki, ei in ((0, 0), (0, 1), (1, 0)):
                nc.tensor.matmul(ps[:, ob:oe], K8[ki][:, w, :, half, :],
                                 enc8[ei][b][:, :, lo:lo + (oe - ob)],
                                 start=first, stop=(w == 2 and ki == 1),
                                 perf_mode=DR)
                first = False
        st = stp.tile([P, 512], BF16, name="cvst")
        # alternate the psum->sbuf staging between Activation and DVE so
        # neither queue's head-of-line blocking can stall the conv psum pool
        # (GPSIMD cannot read PSUM on hardware)
        nc.scalar.copy(st[:], ps[:])
        return st

    eps = [None] * BC
    stages = [[None, None] for _ in range(NCH)]  # stages of batch currently conv'd
    stage_bufs = {}

    def relus_j(b, j, sts):
        # relu(conv + vbias) for both heads/halves; emitted as early as its
        # inputs allow so the DVE never gates the entity-head matmuls
        rs = {}
        for half in range(2):       # half-major: half-1 relus never block
            for v in range(2):      # a half-0 consumer in the DVE queue
                r = rp.tile([P, 512], BF16, name="relu")
                nc.vector.tensor_scalar(r[:], sts[half][:],
                                        vbT[v][:, 0, half, b:b + 1], 0.0,
                                        op0=ADD, op1=MAX)
                if j == 0:
                    nc.vector.tensor_scalar(r[:, 0:1], sts[half][:, 0:1],
                                            vbT[v][:, 1, half, b:b + 1], 0.0,
                                            op0=ADD, op1=MAX)
                if j == NCH - 1:
                    nc.vector.tensor_scalar(r[:, 511:512], sts[half][:, 511:512],
                                            vbT[v][:, 2, half, b:b + 1], 0.0,
                                            op0=ADD, op1=MAX)
                rs[v * 2 + half] = r
        return rs

    def entmm_j(b, j, rs):
        for half in range(2):
            for v in range(2):
                r = rs[v * 2 + half]
                for sc4 in range(4):
                    c = (j * 4 + sc4) * 4 + v * 2
                    nc.tensor.matmul(eps[b][:, c:c + 2],
                                     r[:, sc4 * 128:(sc4 + 1) * 128],
                                     Went[:, half, :],
                                     start=(j == 0 and v == 0 and half == 0
                                            and sc4 == 0),
                                     stop=(j == NCH - 1 and v == 1 and half == 1
                                           and sc4 == 3))

    def ent_j(b, j, sts):
        entmm_j(b, j, relus_j(b, j, sts))

    def ent_flush(b, part=None):
        # eps[b] [128 s, 64 (sc,v,e)] -> transpose -> +bias -> one DMA.
        # part splits the flush in column halves so the tail can overlap.
        lo, n = (0, 64) if part is None else (part * 32, 32)
        esb = ep.tile([P, n], F32, name=f"esb{b}_{part}", bufs=1)
        nc.scalar.copy(esb[:], eps[b][:, lo:lo + n])
        trp = psm.tile([n, P], F32, name=f"trp{b}_{part}", tag="ps")
        nc.tensor.transpose(trp[:], esb[:], id_f32[:])
        trow = ep.tile([n, P], F32, name=f"trow{b}_{part}", bufs=1)
        nc.scalar.activation(trow[:], trp[:], Ident, bias=bent64[lo:lo + n, :])
        ov = out_ap[b:b + 1, R:R + 4 * S].rearrange(
            "o (k c p) -> o c k p", k=4, c=16, p=128)
        dma(out=ov[:, lo // 4:(lo + n) // 4], in_=trow[:])

    def chain(b):
      with tc.high_priority(400):
        eps[b] = pse.tile([P, 64], F32, name=f"eps{b}", tag="eps")
        att = scores_p1(b)
        rsb = scores_p2(b, att)
        mix(b, att, rsb)
        for a in range(3):
            attend_b(a, b)
        t1_col(b)
        vbias_b(0, b)
        vbias_b(1, b)

    def batch_block(b, chain_self=True, chain_next=False):
        """scores/mix/attends/vb interleaved into conv(b) so the PE reaches
        each piece roughly when its DMA dependency lands and the cross-engine
        latencies hide behind conv matmuls."""
        if chain_self:
            eps[b] = pse.tile([P, 64], F32, name=f"eps{b}", tag="eps")
            att = scores_p1(b)
        rsA = [relus_j(b - 1, j, stage_bufs[(b - 1, j)]) for j in (0, 1)]
        stages[0] = [conv_half(b, 0, h) for h in range(2)]
        if chain_self:
            rsb = scores_p2(b, att)
        rsB = [relus_j(b - 1, j, stage_bufs[(b - 1, j)]) for j in (2, 3)]
        stages[1] = [conv_half(b, 1, h) for h in range(2)]
        if chain_self:
            mix(b, att, rsb)
        stages[2] = [conv_half(b, 2, h) for h in range(2)]
        if chain_self:
            for a in range(3):
                attend_b(a, b)
            t1_col(b)
            vbias_b(0, b)
            vbias_b(1, b)
        entmm_j(b - 1, 0, rsA[0])
        entmm_j(b - 1, 1, rsA[1])
        if chain_next:
            bn = b + 1
            eps[bn] = pse.tile([P, 64], F32, name=f"eps{bn}", tag="eps")
            attN = scores_p1(bn)
        stages[3] = [conv_half(b, 3, h) for h in range(2)]
        if chain_next:
            rsbN = scores_p2(bn, attN)
        entmm_j(b - 1, 2, rsB[0])
        entmm_j(b - 1, 3, rsB[1])
        if chain_next:
            mix(bn, attN, rsbN)
        ent_flush(b - 1)
        if chain_next:
            for a in range(3):
                attend_b(a, bn)
            t1_col(bn)
            vbias_b(0, bn)
            vbias_b(1, bn)
        for j in range(NCH):
            del stage_bufs[(b - 1, j)]
            stage_bufs[(b, j)] = stages[j]

    def block6(b=BC - 2):
        """penultimate batch: both remaining attention chains are emitted
        before any of this block's conv staging, so no conv copy ever queues
        behind a chain activation; entity heads then chase the conv."""
        b7 = b + 1
        eps[b7] = pse.tile([P, 64], F32, name=f"eps{b7}", tag="eps")
        att7 = scores_p1(b7)
        rsA = [relus_j(b - 1, j, stage_bufs[(b - 1, j)]) for j in (0, 1)]
        stages[0] = [conv_half(b, 0, h) for h in range(2)]
        rsb7 = scores_p2(b7, att7)
        rsB = [relus_j(b - 1, j, stage_bufs[(b - 1, j)]) for j in (2, 3)]
        stages[1] = [conv_half(b, 1, h) for h in range(2)]
        mix(b7, att7, rsb7)
        for a in range(3):
            attend_b(a, b7)
        t1_col(b7)
        vbias_b(0, b7)
        vbias_b(1, b7)
        t1_flush()
        entmm_j(b - 1, 0, rsA[0])
        entmm_j(b - 1, 1, rsA[1])
        r60 = relus_j(b, 0, stages[0])
        stages[2] = [conv_half(b, 2, h) for h in range(2)]
        entmm_j(b - 1, 2, rsB[0])
        entmm_j(b - 1, 3, rsB[1])
        ent_flush(b - 1)
        r61 = relus_j(b, 1, stages[1])
        entmm_j(b, 0, r60)
        stages[3] = [conv_half(b, 3, h) for h in range(2)]
        entmm_j(b, 1, r61)
        r62 = relus_j(b, 2, stages[2])
        s70 = [conv_half(b7, 0, h) for h in range(2)]
        entmm_j(b, 2, r62)
        r63 = relus_j(b, 3, stages[3])
        s71 = [conv_half(b7, 1, h) for h in range(2)]
        entmm_j(b, 3, r63)
        ent_flush(b)
        r70 = relus_j(b7, 0, s70)
        s72 = [conv_half(b7, 2, h) for h in range(2)]
        entmm_j(b7, 0, r70)
        r71 = relus_j(b7, 1, s71)
        s73 = [conv_half(b7, 3, h) for h in range(2)]
        entmm_j(b7, 1, r71)
        r72 = relus_j(b7, 2, s72)
        entmm_j(b7, 2, r72)
        r73 = relus_j(b7, 3, s73)
        entmm_j(b7, 3, r73)
        ent_flush(b7)

    # ---- emission: PE p-state warmup (tiny matmuls on memset constants, no
    # DMA deps) so the conv runs at full clock from its first matmul ----
    for wi in range(30):
        wps = psm.tile([BC, P], F32, name=f"warm{wi}", tag="ps")
        nc.tensor.matmul(wps[:], ones8[:], onerow_bf[:], start=True, stop=True)

    # ---- conv(b0) interleaved with the LSTM chain ----
    eps[0] = pse.tile([P, 64], F32, name="eps0", tag="eps")
    stages[0] = [conv_half(0, 0, h) for h in range(2)]
    gp = gates(0, lambda ch: h0T[:, ch, :])
    c1 = lstm_nl(0, gp, c0T)
    stages[1] = [conv_half(0, 1, h) for h in range(2)]
    gp = gates(1, lambda ch: hQ[:, ch, 0, :])
    c2 = lstm_nl(1, gp, c1)
    stages[2] = [conv_half(0, 2, h) for h in range(2)]
    gp = gates(2, lambda ch: hQ[:, ch, 1, :])
    lstm_nl(2, gp, c2)
    stages[3] = [conv_half(0, 3, h) for h in range(2)]
    att0 = scores_p1(0)
    rsb0 = scores_p2(0, att0)
    mix(0, att0, rsb0)
    for a in range(3):
        attend_b(a, 0)
    t1_col(0)
    vbias_b(0, 0)
    vbias_b(1, 0)
    for j in range(NCH):
        stage_bufs[(0, j)] = stages[j]

    def t1_flush():
        t1sb = ep.tile([R, BC], F32, name="t1sb")
        nc.scalar.copy(t1sb[:], t1_ps[:])
        t1tr = psm.tile([BC, R], F32, name="t1tr", tag="ps")
        nc.tensor.transpose(t1tr[:], t1sb[:], id_f32[:R, :R])
        t1row = ep.tile([BC, R], F32, name="t1row")
        nc.scalar.copy(t1row[:], t1tr[:])
        dma(out=out_ap[:, 0:R], in_=t1row[:])

    batch_block(1, chain_self=True, chain_next=True)
    batch_block(2, chain_self=False, chain_next=True)
    batch_block(3, chain_self=False, chain_next=True)
    batch_block(4, chain_self=False, chain_next=True)
    batch_block(5, chain_self=False, chain_next=True)
    block6()


def build_nc():
    nc = bacc.Bacc("TRN2", target_bir_lowering=False, debug=False)
    io = {}

    def din(name, shape, dt):
        io[name] = nc.dram_tensor(name, shape, dt, kind="ExternalInput")

    din("e8hi", [BC, 128, 2, S], F8)
    din("e8lo", [BC, 128, 2, S], F8)
    din("enc_sc", [BC, 128, 16, E], BF16)
    din("wblob", [128, WTOT], BF16)
    din("bblob", [1, BTOT], BF16)
    din("w8blob", [128, 2, 3, 2, 2, 128], F8)
    din("bent64", [64, 1], F32)
    din("c0T", [128, 2, BC], F32)
    io["out"] = nc.dram_tensor("out", [BC, R + 4 * S], F32, kind="ExternalOutput")

    with ExitStack() as ctx:
        t = ctx.enter_context(tile.TileContext(nc))
        _emit(ctx, t, nc, io)
    nc.compile()
    return nc


def _pack2(w):  # [256, N] fp32 -> [128, 2, N]
    return np.ascontiguousarray(w.reshape(2, 128, -1).transpose(1, 0, 2))


def prepare_in_maps(inputs):
    bf = ml_dtypes.bfloat16
    f8 = ml_dtypes.float8_e4m3
    enc = np.asarray(inputs["encoder_o"], np.float32)
    enc_bf = enc.astype(bf)
    # [b, p, ch, s] layout: x[b, p, ch, s] = v[b, s, ch*128+p]
    def to_cs(v):
        return np.ascontiguousarray(
            v.transpose(0, 2, 1).reshape(B, 2, 128, S).transpose(0, 2, 1, 3))
    enc_hi = enc.astype(f8)
    enc_lo = (enc - enc_hi.astype(np.float32)).astype(f8)
    e8hi = to_cs(enc_hi)
    e8lo = to_cs(enc_lo)
    W_ih = np.asarray(inputs["W_ih"], np.float32)
    W_hh = np.asarray(inputs["W_hh"], np.float32)
    W_attn = np.asarray(inputs["W_attn"], np.float32)
    kern = np.asarray(inputs["W_conv"], np.float32).transpose(2, 1, 0)  # [3,2E,E]
    Kenc_ = kern[:, :E, :]
    Kv = kern[:, E:, :]
    Kv_i, Kv_f, Kv_l = Kv.sum(0), Kv[1] + Kv[2], Kv[0] + Kv[1]
    # Kenc fp8 hi/lo pack [128, 2, 3, 2, 2, 128]:
    # [p,i,w,ch,half,m] = Khi/lo[w, ch*128+p, half*128+m]
    K_hi = Kenc_.astype(f8)
    K_lo = (Kenc_ - K_hi.astype(np.float32)).astype(f8)
    kp = np.stack([
        k.reshape(3, 2, 128, 2, 128).transpose(2, 0, 1, 3, 4)
        for k in (K_hi, K_lo)], 1)  # [128, 2, 3, 2, 2, 128]
    We = np.stack([np.asarray(inputs["W_ent1"])[0], np.asarray(inputs["W_ent2"])[0]], 1)
    x1 = np.broadcast_to(np.asarray(inputs["sos_emb"])[0], (B, E))
    x2 = np.asarray(inputs["rel_emb"])[np.asarray(inputs["r_in"]).astype(np.int64)]
    idx = np.arange(B)
    k1 = np.asarray(inputs["k1"])[:, 0].astype(np.int64)
    k2 = np.asarray(inputs["k2"])[:, 0].astype(np.int64)
    x3 = enc[idx, k1] + enc[idx, k2]
    X = np.stack([x1, x2, x3], 0).astype(np.float32)      # [3,B,E]
    h0 = np.asarray(inputs["h0"], np.float32)[0]
    c0 = np.asarray(inputs["c0"], np.float32)
    c0 = c0[0] if c0.ndim == 3 else c0                    # [B, E]

    wsh = np.zeros((128, WTOT), np.float32)
    bsh = np.zeros((1, BTOT), np.float32)

    def put(name, arr):                      # arr -> [128, n] block
        o, n = WOFF[name]
        wsh[:, o:o + n] = arr.reshape(128, n)

    def putrow(name, vec):                   # 1-row bias blob entries
        o, n = BOFF[name]
        bsh[0, o:o + n] = vec.ravel()

    put("W_ihT", _pack2(W_ih.T))
    put("W_hhT", _pack2(W_hh.T))
    put("Wa_mT", _pack2(W_attn[:, :E].T))
    put("Wa_qT", _pack2(W_attn[:, E:].T))
    put("Kv_i", _pack2(Kv_i))
    put("Kv_f", _pack2(Kv_f))
    put("Kv_l", _pack2(Kv_l))
    put("W_relT", _pack2(np.asarray(inputs["W_rel"], np.float32).T))
    put("Went", _pack2(We))
    putrow("bias_g", np.asarray(inputs["b_ih"], np.float32)
           + np.asarray(inputs["b_hh"], np.float32))
    putrow("b_attn", np.asarray(inputs["b_attn"], np.float32))
    putrow("b_conv", np.asarray(inputs["b_conv"], np.float32))
    putrow("b_rel", np.asarray(inputs["b_rel"], np.float32))
    be1 = float(np.asarray(inputs["b_ent1"]).ravel()[0])
    be2 = float(np.asarray(inputs["b_ent2"]).ravel()[0])
    bent64 = np.ascontiguousarray(
        np.tile(np.array([be1, be2], np.float32), 32).reshape(64, 1))
    in_maps = []
    for c in range(NCORES):
        sl = slice(c * BC, (c + 1) * BC)
        w = wsh.copy()
        xs = X[:, sl]                                      # [3,BC,E]
        xo, xn = WOFF["xT"]
        w[:, xo:xo + xn] = xs.transpose(2, 0, 1).reshape(
            2, 128, 3, BC).transpose(1, 2, 0, 3).reshape(128, xn)
        ho, hn = WOFF["h0T"]
        w[:, ho:ho + hn] = h0[sl].T.reshape(2, 128, BC).transpose(
            1, 0, 2).reshape(128, hn)
        m = {
            "e8hi": np.ascontiguousarray(e8hi[sl]),
            "e8lo": np.ascontiguousarray(e8lo[sl]),
            "enc_sc": np.ascontiguousarray(
                enc_bf[sl].reshape(BC, 16, 128, E).transpose(0, 2, 1, 3)),
            "wblob": w.astype(bf),
            "bblob": bsh.astype(bf),
            "w8blob": np.ascontiguousarray(kp),
            "bent64": bent64,
            "c0T": np.ascontiguousarray(
                c0[sl].T.reshape(2, 128, BC).transpose(1, 0, 2)),
        }
        in_maps.append(m)
    return in_maps


_NC_CACHE = {}


def get_nc():
    if "nc" not in _NC_CACHE:
        _NC_CACHE["nc"] = build_nc()
    return _NC_CACHE["nc"]


def kernel(**inputs) -> np.ndarray:
    nc = get_nc()
    in_maps = prepare_in_maps(inputs)
    res = run_bass_kernel_spmd(nc, in_maps, core_ids=list(range(NCORES)))
    return np.concatenate([r["out"] for r in res.results], 0).astype(np.float32)


if __name__ == "__main__":
    import jax
    import reference as refmod
    with jax.default_device(jax.devices("cpu")[0]):
        inputs = {k: np.asarray(v) for k, v in refmod.setup_inputs().items()}
        expected = np.asarray(refmod.reference(**inputs))
    actual = kernel(**inputs)
    err = np.abs(actual - expected)
    print("max abs err:", err.max(), "rel:", err.max() / np.abs(expected).max())
